# revision 9
# baseline (speedup 1.0000x reference)
"""InvertedReorg (depth-to-space, slice=2) Trainium2 Bass kernel.

Full input x: (32, 256, 64, 64) f32 -> output (32, 64, 128, 128) f32 with
    y[b, c, s1*64 + h, s2*64 + w] = x[b, s1*128 + s2*64 + c, h, w]
Data-parallel over batch: 4 samples per core.

Hybrid schedule: the job is descriptor-rate bound, not HBM bound. A
direct DRAM->DRAM copy needs one descriptor pair per 256B output chunk
(the s2 interleave granularity); SWDGE generates those at ~223 GB/s but
each HWDGE ring only ~85 GB/s. So:
  - SWDGE (gpsimd) carries _DIRECT_UNITS of the 8 (b, s1) units as
    direct DRAM->DRAM scatters (no SBUF round trip).
  - The HWDGE rings (sync=loads, scalar=stores) pipeline the remaining
    units through SBUF with >=8KB descriptors: load [128, 2048] per s2
    with partition p = 2c + h2 (c = out channel, h2 = row half), DVE
    shuffles [s2, h, w] -> [h, s2, w] within each partition, store
    [128, 4096] lands 32KB-contiguous per output channel.
This splits work across the independent bottlenecks (SWDGE desc-gen vs
SDMA datapath) instead of serializing everything behind one of them.
"""

import numpy as np

_B, _CH, _H, _W = 32, 256, 64, 64
_NCORES = 8
_BPC = _B // _NCORES  # samples per core
_C = _CH // 4  # output channels

# Of the 8 (b, s1) units per core, how many go direct DRAM->DRAM on
# SWDGE (the rest are staged through SBUF on the HWDGE rings).
_DIRECT_UNITS = 3

_cache = {}


def _split_multiwaits(nc, mybir):
    """This walrus build allows one sync-wait command per instruction.
    Tile attaches one wait per dependency, so split the extras into
    same-engine NoOps directly preceding the instruction (the engine
    blocks on each in turn - semantics unchanged)."""
    for f in nc.m.functions:
        for b in f.blocks:
            new_insts = []
            for inst in b.instructions:
                si = inst.sync_info
                if si is not None and len(si.on_wait) > 1:
                    for w in si.on_wait[:-1]:
                        new_insts.append(
                            mybir.InstNoOp(
                                name=f"I-{nc.next_id()}",
                                engine=inst.engine,
                                ins=[],
                                outs=[],
                                sync_info=mybir.SyncInfo(on_wait=[w], on_update=[]),
                            )
                        )
                    inst.sync_info = mybir.SyncInfo(
                        on_wait=[si.on_wait[-1]], on_update=list(si.on_update)
                    )
                new_insts.append(inst)
            b.instructions = new_insts


def _build(split_multiwaits=True):
    from concourse import bass, mybir, tile

    nc = bass.Bass()
    x = nc.declare_dram_parameter(
        "x", [_BPC, _CH, _H, _W], mybir.dt.float32, isOutput=False
    )
    y = nc.declare_dram_parameter(
        "y", [_BPC, _C, 2 * _H, 2 * _W], mybir.dt.float32, isOutput=True
    )

    # Staged-path views. Partition p = 2c + h2 (h2 = which 32-row half).
    # Load src per (b, s1, s2): contiguous 1MB = 128 rows x 8KB.
    xr = x.rearrange(
        "b (s1 s2 c) (h2 hh) w -> b s1 s2 (c h2) (hh w)", s1=2, s2=2, h2=2
    )
    # Store dst per (b, s1): rows (c, h2) of 16KB; 32KB contiguous per c.
    yr = y.rearrange(
        "b c (s1 h2 h) (s2 w) -> b s1 c h2 (h s2 w)", s1=2, h2=2, s2=2
    )

    # Direct-path views (src: 64 channel maps contiguous; dst: 64x64
    # rows of 256B at stride 512B).
    xd = x.rearrange("b (s1 s2 c) h w -> b s1 s2 c h w", s1=2, s2=2)
    yd = y.rearrange("b c (s1 hh) (s2 w) -> b s1 s2 c hh w", s1=2, s2=2)

    units = [(b, s1) for b in range(_BPC) for s1 in range(2)]
    # 3 direct units spread through issue order on SWDGE; the last unit
    # is direct on the HWDGE rings, issued AFTER their staged work
    # (trailing, so it never stalls the staged pipeline): 1.5MB on the
    # load ring (which otherwise idles first), 0.5MB on the store ring.
    direct_gp = {0, 3, 5}
    direct_ring = {7}

    with tile.TileContext(nc) as tc:
        with (
            tc.tile_pool(name="L", bufs=4) as lp,
            tc.tile_pool(name="S", bufs=4) as sp,
        ):
            for i, (b, s1) in enumerate(units):
                if i in direct_gp:
                    for s2 in range(2):
                        nc.gpsimd.dma_start(out=yd[b, s1, s2], in_=xd[b, s1, s2])
                elif i in direct_ring:
                    continue
                else:
                    L = lp.tile([128, 4096], mybir.dt.float32)
                    S = sp.tile([128, 4096], mybir.dt.float32)
                    nc.sync.dma_start(out=L[:, 0:2048], in_=xr[b, s1, 0])
                    nc.sync.dma_start(out=L[:, 2048:4096], in_=xr[b, s1, 1])
                    # Split copy+store by partition half so the final
                    # store can start before the whole copy is done.
                    Sv = S.rearrange("p (h s2 w) -> p s2 h w", h=32, s2=2, w=64)
                    Lv = L.rearrange("p (s2 h w) -> p s2 h w", s2=2, h=32, w=64)
                    yv = yr[b, s1]
                    nc.vector.tensor_copy(out=Sv[0:64], in_=Lv[0:64])
                    nc.scalar.dma_start(out=yv[0:32], in_=S[0:64, :])
                    nc.vector.tensor_copy(out=Sv[64:128], in_=Lv[64:128])
                    nc.scalar.dma_start(out=yv[32:64], in_=S[64:128, :])
            for i, (b, s1) in enumerate(units):
                if i not in direct_ring:
                    continue
                # s2=0 whole + half of s2=1 trailing on the load ring
                # (idles first); the last 0.5MB on SWDGE.
                nc.sync.dma_start(out=yd[b, s1, 0], in_=xd[b, s1, 0])
                nc.sync.dma_start(
                    out=yd[b, s1, 1][0:32], in_=xd[b, s1, 1][0:32]
                )
                nc.gpsimd.dma_start(
                    out=yd[b, s1, 1][32:64], in_=xd[b, s1, 1][32:64]
                )
    if split_multiwaits:
        _split_multiwaits(nc, mybir)
    return nc


def kernel(x: np.ndarray) -> np.ndarray:
    from concourse.bass_utils import run_bass_kernel_spmd

    if "nc" not in _cache:
        _cache["nc"] = _build()
    nc = _cache["nc"]

    x = np.ascontiguousarray(np.asarray(x), dtype=np.float32)
    in_maps = [{"x": x[i * _BPC : (i + 1) * _BPC]} for i in range(_NCORES)]
    res = run_bass_kernel_spmd(nc, in_maps, list(range(_NCORES)))
    return np.concatenate([res.results[i]["y"] for i in range(_NCORES)], axis=0)


# revision 12
# speedup vs baseline: 1.0565x; 1.0565x over previous
"""InvertedReorg (depth-to-space, slice=2) Trainium2 Bass kernel.

Full input x: (32, 256, 64, 64) f32 -> output (32, 64, 128, 128) f32 with
    y[b, c, s1*64 + h, s2*64 + w] = x[b, s1*128 + s2*64 + c, h, w]
Data-parallel over batch: 4 samples per core.

Hybrid schedule: the job is descriptor-rate bound, not HBM bound. A
direct DRAM->DRAM copy needs one descriptor pair per 256B output chunk
(the s2 interleave granularity); SWDGE generates those at ~130-240 GB/s
but each HWDGE ring only ~85 GB/s, while the 16 SDMA engines aggregate
~400-420 GB/s and staged traffic crosses them twice. Balancing those
bottlenecks (measured over ~8 profiled variants, all-core exec max
107.4us baseline -> 86.4us):
  - 3 of the 8 (b, s1) units go direct DRAM->DRAM on SWDGE (gpsimd);
    one engine pass per byte, desc-gen on the Q7 CounterMachine.
  - 4 units pipeline through SBUF on the HWDGE rings with >=8KB
    descriptors: load [128, 2048] per s2 with partition p = 2c + h2
    (c = out channel, h2 = row half), DVE shuffles [s2, h, w] ->
    [h, s2, w] within each partition (~2.3us/unit, 2x f32 mode), store
    [128, 4096] lands 32KB-contiguous per output channel.
  - The last unit is trailing small-desc direct work on the HWDGE
    rings (1.5MB sync / 0.5MB scalar), issued after their staged DMAs
    so all three queues drain at about the same time.
"""

import numpy as np

_B, _CH, _H, _W = 32, 256, 64, 64
_NCORES = 8
_BPC = _B // _NCORES  # samples per core
_C = _CH // 4  # output channels

_cache = {}


def _split_multiwaits(nc, mybir):
    """This walrus build allows one sync-wait command per instruction.
    Tile attaches one wait per dependency, so split the extras into
    same-engine NoOps directly preceding the instruction (the engine
    blocks on each in turn - semantics unchanged)."""
    for f in nc.m.functions:
        for b in f.blocks:
            new_insts = []
            for inst in b.instructions:
                si = inst.sync_info
                if si is not None and len(si.on_wait) > 1:
                    for w in si.on_wait[:-1]:
                        new_insts.append(
                            mybir.InstNoOp(
                                name=f"I-{nc.next_id()}",
                                engine=inst.engine,
                                ins=[],
                                outs=[],
                                sync_info=mybir.SyncInfo(on_wait=[w], on_update=[]),
                            )
                        )
                    inst.sync_info = mybir.SyncInfo(
                        on_wait=[si.on_wait[-1]], on_update=list(si.on_update)
                    )
                new_insts.append(inst)
            b.instructions = new_insts


def _build(split_multiwaits=True):
    from concourse import bass, mybir, tile

    nc = bass.Bass()
    x = nc.declare_dram_parameter(
        "x", [_BPC, _CH, _H, _W], mybir.dt.float32, isOutput=False
    )
    y = nc.declare_dram_parameter(
        "y", [_BPC, _C, 2 * _H, 2 * _W], mybir.dt.float32, isOutput=True
    )

    # Staged-path views. Partition p = 2c + h2 (h2 = which 32-row half).
    # Load src per (b, s1, s2): contiguous 1MB = 128 rows x 8KB.
    xr = x.rearrange(
        "b (s1 s2 c) (h2 hh) w -> b s1 s2 (c h2) (hh w)", s1=2, s2=2, h2=2
    )
    # Store dst per (b, s1): rows (c, h2) of 16KB; 32KB contiguous per c.
    yr = y.rearrange(
        "b c (s1 h2 h) (s2 w) -> b s1 c h2 (h s2 w)", s1=2, h2=2, s2=2
    )

    # Direct-path views (src: 64 channel maps contiguous; dst: 64x64
    # rows of 256B at stride 512B).
    xd = x.rearrange("b (s1 s2 c) h w -> b s1 s2 c h w", s1=2, s2=2)
    yd = y.rearrange("b c (s1 hh) (s2 w) -> b s1 s2 c hh w", s1=2, s2=2)

    units = [(b, s1) for b in range(_BPC) for s1 in range(2)]
    # 3 direct units spread through issue order on SWDGE; unit 7 is
    # direct on the HWDGE rings, issued AFTER their staged work
    # (trailing, so it never stalls the staged pipeline): 1.5MB on the
    # load ring (which otherwise idles first), 0.5MB on the store ring.
    direct_gp = {0, 3, 5}
    direct_ring = {7}

    with tile.TileContext(nc) as tc:
        with (
            tc.tile_pool(name="L", bufs=4) as lp,
            tc.tile_pool(name="S", bufs=4) as sp,
        ):
            for i, (b, s1) in enumerate(units):
                if i in direct_gp:
                    for s2 in range(2):
                        nc.gpsimd.dma_start(out=yd[b, s1, s2], in_=xd[b, s1, s2])
                elif i in direct_ring:
                    continue
                else:
                    L = lp.tile([128, 4096], mybir.dt.float32)
                    S = sp.tile([128, 4096], mybir.dt.float32)
                    nc.sync.dma_start(out=L[:, 0:2048], in_=xr[b, s1, 0])
                    nc.sync.dma_start(out=L[:, 2048:4096], in_=xr[b, s1, 1])
                    nc.vector.tensor_copy(
                        out=S.rearrange("p (h s2 w) -> p s2 h w", h=32, s2=2, w=64),
                        in_=L.rearrange("p (s2 h w) -> p s2 h w", s2=2, h=32, w=64),
                    )
                    nc.scalar.dma_start(out=yr[b, s1], in_=S[:, :])
            for i, (b, s1) in enumerate(units):
                if i not in direct_ring:
                    continue
                # s2=0 whole + first half of s2=1 on sync; rest on scalar.
                nc.sync.dma_start(out=yd[b, s1, 0], in_=xd[b, s1, 0])
                nc.sync.dma_start(
                    out=yd[b, s1, 1][0:32], in_=xd[b, s1, 1][0:32]
                )
                nc.scalar.dma_start(
                    out=yd[b, s1, 1][32:64], in_=xd[b, s1, 1][32:64]
                )
    if split_multiwaits:
        _split_multiwaits(nc, mybir)
    return nc


def kernel(x: np.ndarray) -> np.ndarray:
    from concourse.bass_utils import run_bass_kernel_spmd

    if "nc" not in _cache:
        _cache["nc"] = _build()
    nc = _cache["nc"]

    x = np.ascontiguousarray(np.asarray(x), dtype=np.float32)
    in_maps = [{"x": x[i * _BPC : (i + 1) * _BPC]} for i in range(_NCORES)]
    res = run_bass_kernel_spmd(nc, in_maps, list(range(_NCORES)))
    return np.concatenate([res.results[i]["y"] for i in range(_NCORES)], axis=0)


# revision 13
# speedup vs baseline: 1.1795x; 1.1165x over previous
"""InvertedReorg (depth-to-space, slice=2) Trainium2 Bass kernel.

Full input x: (32, 256, 64, 64) f32 -> output (32, 64, 128, 128) f32 with
    y[b, c, s1*64 + h, s2*64 + w] = x[b, s1*128 + s2*64 + c, h, w]
Data-parallel over batch: 4 samples per core.

Hybrid schedule: the job is descriptor-rate bound, not HBM bound. A
direct DRAM->DRAM copy needs one descriptor pair per 256B output chunk
(the s2 interleave granularity); SWDGE generates those at ~130-240 GB/s
but each HWDGE ring only ~85 GB/s, while the 16 SDMA engines aggregate
~400-420 GB/s and staged traffic crosses them twice. Balancing those
bottlenecks (measured over ~8 profiled variants, all-core exec max
107.4us baseline -> 86.4us):
  - 3 of the 8 (b, s1) units go direct DRAM->DRAM on SWDGE (gpsimd);
    one engine pass per byte, desc-gen on the Q7 CounterMachine.
  - 4 units pipeline through SBUF on the HWDGE rings with >=8KB
    descriptors: load [128, 2048] per s2 with partition p = 2c + h2
    (c = out channel, h2 = row half), DVE shuffles [s2, h, w] ->
    [h, s2, w] within each partition (~2.3us/unit, 2x f32 mode), store
    [128, 4096] lands 32KB-contiguous per output channel.
  - The last unit is trailing small-desc direct work on the HWDGE
    rings (1.5MB sync / 0.5MB scalar), issued after their staged DMAs
    so all three queues drain at about the same time.
"""

import numpy as np

_B, _CH, _H, _W = 32, 256, 64, 64
_NCORES = 8
_BPC = _B // _NCORES  # samples per core
_C = _CH // 4  # output channels

_cache = {}


def _split_multiwaits(nc, mybir):
    """This walrus build allows one sync-wait command per instruction.
    Tile attaches one wait per dependency, so split the extras into
    same-engine NoOps directly preceding the instruction (the engine
    blocks on each in turn - semantics unchanged)."""
    for f in nc.m.functions:
        for b in f.blocks:
            new_insts = []
            for inst in b.instructions:
                si = inst.sync_info
                if si is not None and len(si.on_wait) > 1:
                    for w in si.on_wait[:-1]:
                        new_insts.append(
                            mybir.InstNoOp(
                                name=f"I-{nc.next_id()}",
                                engine=inst.engine,
                                ins=[],
                                outs=[],
                                sync_info=mybir.SyncInfo(on_wait=[w], on_update=[]),
                            )
                        )
                    inst.sync_info = mybir.SyncInfo(
                        on_wait=[si.on_wait[-1]], on_update=list(si.on_update)
                    )
                new_insts.append(inst)
            b.instructions = new_insts


def _build(split_multiwaits=True):
    from concourse import bass, mybir, tile

    nc = bass.Bass()
    x = nc.declare_dram_parameter(
        "x", [_BPC, _CH, _H, _W], mybir.dt.float32, isOutput=False
    )
    y = nc.declare_dram_parameter(
        "y", [_BPC, _C, 2 * _H, 2 * _W], mybir.dt.float32, isOutput=True
    )

    # Staged-path views. Partition p = 2c + h2 (h2 = which 32-row half).
    # Load src per (b, s1, s2): contiguous 1MB = 128 rows x 8KB.
    xr = x.rearrange(
        "b (s1 s2 c) (h2 hh) w -> b s1 s2 (c h2) (hh w)", s1=2, s2=2, h2=2
    )
    # Store dst per (b, s1): rows (c, h2) of 16KB; 32KB contiguous per c.
    yr = y.rearrange(
        "b c (s1 h2 h) (s2 w) -> b s1 c h2 (h s2 w)", s1=2, h2=2, s2=2
    )

    # Direct-path views (src: 64 channel maps contiguous; dst: 64x64
    # rows of 256B at stride 512B).
    xd = x.rearrange("b (s1 s2 c) h w -> b s1 s2 c h w", s1=2, s2=2)
    yd = y.rearrange("b c (s1 hh) (s2 w) -> b s1 s2 c hh w", s1=2, s2=2)

    units = [(b, s1) for b in range(_BPC) for s1 in range(2)]
    # 3 direct units spread through issue order on SWDGE; unit 7 is
    # direct on the HWDGE rings, issued AFTER their staged work
    # (trailing, so it never stalls the staged pipeline): 1.5MB on the
    # load ring (which otherwise idles first), 0.5MB on the store ring.
    direct_gp = {0, 3, 5}
    direct_ring = {7}

    from concourse.tile_rust import add_dep_helper

    with tile.TileContext(nc) as tc:
        with (
            tc.tile_pool(name="L", bufs=4) as lp,
            tc.tile_pool(name="S", bufs=4) as sp,
        ):
            last_load = last_store = None
            for i, (b, s1) in enumerate(units):
                if i in direct_gp:
                    for s2 in range(2):
                        nc.gpsimd.dma_start(out=yd[b, s1, s2], in_=xd[b, s1, s2])
                elif i in direct_ring:
                    continue
                else:
                    L = lp.tile([128, 4096], mybir.dt.float32)
                    S = sp.tile([128, 4096], mybir.dt.float32)
                    nc.sync.dma_start(out=L[:, 0:2048], in_=xr[b, s1, 0])
                    last_load = nc.sync.dma_start(
                        out=L[:, 2048:4096], in_=xr[b, s1, 1]
                    )
                    nc.vector.tensor_copy(
                        out=S.rearrange("p (h s2 w) -> p s2 h w", h=32, s2=2, w=64),
                        in_=L.rearrange("p (s2 h w) -> p s2 h w", s2=2, h=32, w=64),
                    )
                    last_store = nc.scalar.dma_start(out=yr[b, s1], in_=S[:, :])
            for i, (b, s1) in enumerate(units):
                if i not in direct_ring:
                    continue
                # s2=0 whole + first half of s2=1 on sync; rest on scalar.
                # Ordering-only deps keep the scheduler's gap-filler from
                # dispatching these small-desc DMAs ahead of the staged
                # work on the same ring (measured: stores delayed to 39.6us
                # by a front-run 0.5MB direct).
                d0 = nc.sync.dma_start(out=yd[b, s1, 0], in_=xd[b, s1, 0])
                d1 = nc.sync.dma_start(
                    out=yd[b, s1, 1][0:32], in_=xd[b, s1, 1][0:32]
                )
                d2 = nc.scalar.dma_start(
                    out=yd[b, s1, 1][32:64], in_=xd[b, s1, 1][32:64]
                )
                add_dep_helper(
                    d0.ins, last_load.ins, sync=False, reason="trail loads"
                )
                add_dep_helper(
                    d1.ins, last_load.ins, sync=False, reason="trail loads"
                )
                add_dep_helper(
                    d2.ins, last_store.ins, sync=False, reason="trail stores"
                )
    if split_multiwaits:
        _split_multiwaits(nc, mybir)
    return nc


def kernel(x: np.ndarray) -> np.ndarray:
    from concourse.bass_utils import run_bass_kernel_spmd

    if "nc" not in _cache:
        _cache["nc"] = _build()
    nc = _cache["nc"]

    x = np.ascontiguousarray(np.asarray(x), dtype=np.float32)
    in_maps = [{"x": x[i * _BPC : (i + 1) * _BPC]} for i in range(_NCORES)]
    res = run_bass_kernel_spmd(nc, in_maps, list(range(_NCORES)))
    return np.concatenate([res.results[i]["y"] for i in range(_NCORES)], axis=0)


# revision 14
# speedup vs baseline: 1.1819x; 1.0020x over previous
"""InvertedReorg (depth-to-space, slice=2) Trainium2 Bass kernel.

Full input x: (32, 256, 64, 64) f32 -> output (32, 64, 128, 128) f32 with
    y[b, c, s1*64 + h, s2*64 + w] = x[b, s1*128 + s2*64 + c, h, w]
Data-parallel over batch: 4 samples per core.

Hybrid schedule: the job is descriptor-rate bound, not HBM bound. A
direct DRAM->DRAM copy needs one descriptor pair per 256B output chunk
(the s2 interleave granularity); SWDGE generates those at ~130-240 GB/s
but each HWDGE ring only ~85 GB/s, while the 16 SDMA engines aggregate
~400-420 GB/s and staged traffic crosses them twice. Balancing those
bottlenecks (measured over ~8 profiled variants, all-core exec max
107.4us baseline -> 86.4us):
  - 3 of the 8 (b, s1) units go direct DRAM->DRAM on SWDGE (gpsimd);
    one engine pass per byte, desc-gen on the Q7 CounterMachine.
  - 4 units pipeline through SBUF on the HWDGE rings with >=8KB
    descriptors: load [128, 2048] per s2 with partition p = 2c + h2
    (c = out channel, h2 = row half), DVE shuffles [s2, h, w] ->
    [h, s2, w] within each partition (~2.3us/unit, 2x f32 mode), store
    [128, 4096] lands 32KB-contiguous per output channel.
  - The last unit is trailing small-desc direct work on the HWDGE
    rings (1.5MB sync / 0.5MB scalar), issued after their staged DMAs
    so all three queues drain at about the same time.
"""

import numpy as np

_B, _CH, _H, _W = 32, 256, 64, 64
_NCORES = 8
_BPC = _B // _NCORES  # samples per core
_C = _CH // 4  # output channels

_cache = {}


def _split_multiwaits(nc, mybir):
    """This walrus build allows one sync-wait command per instruction.
    Tile attaches one wait per dependency, so split the extras into
    same-engine NoOps directly preceding the instruction (the engine
    blocks on each in turn - semantics unchanged)."""
    for f in nc.m.functions:
        for b in f.blocks:
            new_insts = []
            for inst in b.instructions:
                si = inst.sync_info
                if si is not None and len(si.on_wait) > 1:
                    for w in si.on_wait[:-1]:
                        new_insts.append(
                            mybir.InstNoOp(
                                name=f"I-{nc.next_id()}",
                                engine=inst.engine,
                                ins=[],
                                outs=[],
                                sync_info=mybir.SyncInfo(on_wait=[w], on_update=[]),
                            )
                        )
                    inst.sync_info = mybir.SyncInfo(
                        on_wait=[si.on_wait[-1]], on_update=list(si.on_update)
                    )
                new_insts.append(inst)
            b.instructions = new_insts


def _build(split_multiwaits=True):
    from concourse import bass, mybir, tile

    nc = bass.Bass()
    x = nc.declare_dram_parameter(
        "x", [_BPC, _CH, _H, _W], mybir.dt.float32, isOutput=False
    )
    y = nc.declare_dram_parameter(
        "y", [_BPC, _C, 2 * _H, 2 * _W], mybir.dt.float32, isOutput=True
    )

    # Staged-path views. Partition p = 2c + h2 (h2 = which 32-row half).
    # Load src per (b, s1, s2): contiguous 1MB = 128 rows x 8KB.
    xr = x.rearrange(
        "b (s1 s2 c) (h2 hh) w -> b s1 s2 (c h2) (hh w)", s1=2, s2=2, h2=2
    )
    # Store dst per (b, s1): rows (c, h2) of 16KB; 32KB contiguous per c.
    yr = y.rearrange(
        "b c (s1 h2 h) (s2 w) -> b s1 c h2 (h s2 w)", s1=2, h2=2, s2=2
    )

    # Direct-path views (src: 64 channel maps contiguous; dst: 64x64
    # rows of 256B at stride 512B).
    xd = x.rearrange("b (s1 s2 c) h w -> b s1 s2 c h w", s1=2, s2=2)
    yd = y.rearrange("b c (s1 hh) (s2 w) -> b s1 s2 c hh w", s1=2, s2=2)

    units = [(b, s1) for b in range(_BPC) for s1 in range(2)]
    # 3 direct units spread through issue order on SWDGE; unit 7 is
    # direct on the HWDGE rings, issued AFTER their staged work
    # (trailing, so it never stalls the staged pipeline): 1.5MB on the
    # load ring (which otherwise idles first), 0.5MB on the store ring.
    direct_gp = {0, 3, 5}
    direct_ring = {7}

    with tile.TileContext(nc) as tc:
        with (
            tc.tile_pool(name="L", bufs=4) as lp,
            tc.tile_pool(name="S", bufs=4) as sp,
        ):
            for i, (b, s1) in enumerate(units):
                if i in direct_gp:
                    for s2 in range(2):
                        nc.gpsimd.dma_start(out=yd[b, s1, s2], in_=xd[b, s1, s2])
                elif i in direct_ring:
                    continue
                else:
                    L = lp.tile([128, 4096], mybir.dt.float32)
                    S = sp.tile([128, 4096], mybir.dt.float32)
                    nc.sync.dma_start(out=L[:, 0:2048], in_=xr[b, s1, 0])
                    nc.sync.dma_start(out=L[:, 2048:4096], in_=xr[b, s1, 1])
                    nc.vector.tensor_copy(
                        out=S.rearrange("p (h s2 w) -> p s2 h w", h=32, s2=2, w=64),
                        in_=L.rearrange("p (s2 h w) -> p s2 h w", s2=2, h=32, w=64),
                    )
                    nc.scalar.dma_start(out=yr[b, s1], in_=S[:, :])
            for i, (b, s1) in enumerate(units):
                if i not in direct_ring:
                    continue
                # s2=0 whole + first half of s2=1 on sync; rest on scalar.
                # (Scheduler-level ordering deps to push these after the
                # staged work were tried and measured zero-sum: the ring
                # DGE's small-desc generation rate, not dispatch order,
                # bounds when this traffic completes.)
                nc.sync.dma_start(out=yd[b, s1, 0], in_=xd[b, s1, 0])
                nc.sync.dma_start(
                    out=yd[b, s1, 1][0:32], in_=xd[b, s1, 1][0:32]
                )
                nc.scalar.dma_start(
                    out=yd[b, s1, 1][32:64], in_=xd[b, s1, 1][32:64]
                )
    if split_multiwaits:
        _split_multiwaits(nc, mybir)
    return nc


def kernel(x: np.ndarray) -> np.ndarray:
    from concourse.bass_utils import run_bass_kernel_spmd

    if "nc" not in _cache:
        _cache["nc"] = _build()
    nc = _cache["nc"]

    x = np.ascontiguousarray(np.asarray(x), dtype=np.float32)
    in_maps = [{"x": x[i * _BPC : (i + 1) * _BPC]} for i in range(_NCORES)]
    res = run_bass_kernel_spmd(nc, in_maps, list(range(_NCORES)))
    return np.concatenate([res.results[i]["y"] for i in range(_NCORES)], axis=0)
